# revision 14
# baseline (speedup 1.0000x reference)
"""Trainium2 Bass kernel for nn_CausalTransformer_71906342469950 (2x1024 tokens,
H=768, 4 layers, 12 heads, FF=3072 with top-307 router sparsity, V=32000).

Sharding: token-zigzag over 8 cores. Cores 0-3 own sequence 0, cores 4-7 own
sequence 1. Within a 4-core group the sequence is split into 8 chunks of 128
tokens; core c owns chunks (c, 7-c), balancing causal-attention cost. Weights
are replicated and streamed; K/V (and the final hiddens for the vocab-sharded
LM head) are all-gathered per group.

Numerics: every matmul feeding the residual stream runs in true fp32 — the
top-k router makes the network chaotically sensitive, so bf16/f32r there blows
up the final error (measured 8-19% rel err vs 0.5% for f32). Only layer-3
W2 and the LM head use float32r (bf16-speed, ~2^-13 error) since nothing after
them re-enters a router. Top-k is an exact 24-step per-token bisection on the
fp32 router logits. LN gains/biases and all linear biases are folded into the
weights host-side (bias lives in an extra contraction row, driven by a
constant-one row in the transposed activations).
"""

import numpy as np

import concourse.bacc as bacc
import concourse.mybir as mybir
import concourse.tile as tile
from concourse.bass_utils import run_bass_kernel_spmd
from concourse.masks import make_identity

F32 = mybir.dt.float32
F32R = mybir.dt.float32r
BF16 = mybir.dt.bfloat16
AFT = mybir.ActivationFunctionType
ALU = mybir.AluOpType

import os as _os
V, H, L, NH, HD, FF = 32000, 768, 4, 12, 64, 3072
L = int(_os.environ.get("KLAYERS", L))
KNO_ATTN = bool(int(_os.environ.get("KNO_ATTN", "0")))
KNO_GATHER = bool(int(_os.environ.get("KNO_GATHER", "0")))
KNO_MLP = bool(int(_os.environ.get("KNO_MLP", "0")))
KNO_TOPK = bool(int(_os.environ.get("KNO_TOPK", "0")))
KNO_W12 = bool(int(_os.environ.get("KNO_W12", "0")))
TOPK = 307
B, S = 2, 1024
P = 128
KS = H // P              # 6
KSP = KS + 1             # + bias row subtile
FFKS = FF // P           # 24
NCHUNK = 8
NCORE = 8
GROUPS = [[0, 1, 2, 3], [4, 5, 6, 7]]
VS = V // 4              # 8000
NEG = -10000.0
EPS = 1e-5
N_BISECT = 24
BIS_LO, BIS_HI = -2.0, 4.0
NFFB = 4                 # ff blocks of 6 subtiles (768 cols) each

CHUNK_OF = [(c, 7 - c) for c in range(4)]
POS_OF = {}
for _c in range(4):
    POS_OF[_c] = (_c, 0)
    POS_OF[7 - _c] = (_c, 1)


def _build_nc():
    nc = bacc.Bacc("TRN2", target_bir_lowering=False)

    x0_d = nc.dram_tensor("x0", [2, P, H], F32, kind="ExternalInput")
    ambvis_d = nc.dram_tensor("ambvis", [P, NCHUNK, 2], F32, kind="ExternalInput")
    dflag_d = nc.dram_tensor("dflag", [P, NCHUNK, 2], F32, kind="ExternalInput")
    w_in = {}
    for l in range(L):
        for nm, shape, dt in [
            ("wq", [KSP * P, H], F32), ("wk", [KSP * P, H], F32),
            ("wv", [KSP * P, H], F32), ("wo", [KSP * P, H], F32),
            ("w1", [KSP * P, FF], F32),
            ("wr", [KSP * P, FF], F32),
            ("w2", [(FFKS + 1) * P, H], F32R if l == L - 1 else F32),
        ]:
            w_in[(nm, l)] = nc.dram_tensor(f"{nm}{l}", shape, dt, kind="ExternalInput")
    wlm_d = nc.dram_tensor("wlm", [KSP * P, VS], F32R, kind="ExternalInput")
    out_d = nc.dram_tensor("logits", [NCHUNK * P, VS], F32, kind="ExternalOutput")

    kT_loc = nc.dram_tensor("kT_loc", [H, 2 * P], F32)
    kT_gat = nc.dram_tensor("kT_gat", [4, H, 2 * P], F32)
    v_loc = nc.dram_tensor("v_loc", [2 * P, NH, HD + 1], F32)
    v_gat = nc.dram_tensor("v_gat", [4, 2 * P, NH, HD + 1], F32)
    xfT_loc = nc.dram_tensor("xfT_loc", [H, 2 * P], F32R)
    xfT_gat = nc.dram_tensor("xfT_gat", [4, H, 2 * P], F32R)

    from contextlib import ExitStack
    with tile.TileContext(nc) as tc, ExitStack() as ctx:
        def pool(name, bufs, space="SBUF"):
            return ctx.enter_context(tc.tile_pool(name=name, bufs=bufs, space=space))
        cst = pool("cst", 1); resid = pool("resid", 1); hTp = pool("hTp", 1)
        htmp = pool("htmp", 2); qTp = pool("qTp", 1); kvp = pool("kvp", 1)
        oTp = pool("oTp", 1); etp = pool("etp", 2); wst = pool("wst", 2)
        hidp = pool("hidp", 2); rrp = pool("rrp", 1); scrp = pool("scrp", 1)
        hidTp = pool("hidTp", 2); smallp = pool("smallp", 4); lop = pool("lop", 1)
        w2sbp = pool("w2sbp", 1)
        psMM = pool("psMM", 2, "PSUM"); psSC = pool("psSC", 2, "PSUM")
        psOT = pool("psOT", 2, "PSUM"); psTR = pool("psTR", 2, "PSUM")
        if True:
            ident = cst.tile([P, P], F32)
            make_identity(nc, ident)
            causal = cst.tile([P, P], F32)  # [k, q] -> NEG where k > q
            causal_i = cst.tile([P, P], mybir.dt.int32)
            nc.gpsimd.iota(causal_i[:], pattern=[[-1, P]], base=0,
                           channel_multiplier=1)
            nc.vector.tensor_scalar(causal[:], causal_i[:], 0.5, None, ALU.is_gt)
            nc.vector.tensor_scalar_mul(causal[:], causal[:], NEG)
            ambvis = cst.tile([P, NCHUNK, 2], F32)
            nc.sync.dma_start(ambvis[:], ambvis_d[:])
            dflag = cst.tile([P, NCHUNK, 2], F32)
            nc.sync.dma_start(dflag[:], dflag_d[:])
            ones_col = cst.tile([P, 1], F32)
            nc.vector.memset(ones_col[:], 1.0)
            ones_blk32 = cst.tile([P, 1, 2 * P], F32)
            nc.vector.memset(ones_blk32[:], 0.0)
            nc.vector.memset(ones_blk32[0:1], 1.0)
            ones_blkr = cst.tile([P, 1, 2 * P], F32R)
            nc.vector.tensor_copy(ones_blkr[:], ones_blk32[:])

            x_sb = resid.tile([P, 2, H], F32)
            nc.sync.dma_start(x_sb[:], x0_d.rearrange("t p h -> p t h"))

            # denominator ones-columns of v_loc, written once
            for t in range(2):
                for head in range(NH):
                    nc.sync.dma_start(
                        v_loc.rearrange("(t p) n d -> p t n d", p=P)[:, t, head, HD:HD + 1],
                        ones_col[:])

            def layernorm(dst, src):
                stats = smallp.tile([P, 2, 6], F32, tag="ln_stats")
                aggr = smallp.tile([P, 2], F32, tag="ln_aggr")
                nc.vector.bn_stats(stats[:, 0], src[:, 0:H // 2])
                nc.vector.bn_stats(stats[:, 1], src[:, H // 2:])
                nc.vector.bn_aggr(aggr[:], stats[:])
                rs = smallp.tile([P, 1], F32, tag="ln_rs")
                nc.vector.tensor_scalar_add(rs[:], aggr[:, 1:2], EPS)
                nc.scalar.activation(rs[:], rs[:], AFT.Sqrt)
                nc.vector.reciprocal(rs[:], rs[:])
                nc.vector.scalar_tensor_tensor(
                    out=dst, in0=src, scalar=aggr[:, 0:1],
                    in1=rs[:, 0:1].to_broadcast(list(dst.shape)),
                    op0=ALU.subtract, op1=ALU.mult)

            def ln_transpose(tag_dt=F32):
                """LN both chunks of x -> transposed [P, KSP, 256] with ones row."""
                hT = hTp.tile([P, KSP, 2 * P], tag_dt, tag="hT")
                nc.vector.tensor_copy(hT[:, KS], ones_blk32[:, 0])
                for t in range(2):
                    h_tmp = htmp.tile([P, H], F32, tag="h_tmp")
                    layernorm(h_tmp[:], x_sb[:, t])
                    for ks in range(KS):
                        pt = psTR.tile([P, P], F32, tag="tr")
                        nc.tensor.transpose(pt[:], h_tmp[:, ks * P:(ks + 1) * P], ident)
                        nc.any.tensor_copy(hT[:, ks, t * P:(t + 1) * P], pt[:])
                return hT

            def load_w(name, l, cols, dt=F32):
                wt = wst.tile([P, KSP, cols.stop - cols.start], dt, tag="wst")
                nc.sync.dma_start(
                    wt[:],
                    w_in[(name, l)].rearrange("(o p) m -> p o m", p=P)[:, :, cols])
                return wt

            for l in range(L):
                tail = (l == L - 1)
                # ---------- LN1 + transpose ----------
                hT = ln_transpose()

                # ---------- Q^T ----------
                qT = qTp.tile([P, KS, 2 * P], F32, tag="qT")
                for half in range(2):
                    wt = load_w("wq", l, slice(half * 384, (half + 1) * 384))
                    for hb3 in range(3):   # 3 blocks of 128 within this 384
                        hb = half * 3 + hb3
                        pt = psMM.tile([P, 2 * P], F32, tag="mm")
                        for ks in range(KSP):
                            nc.tensor.matmul(
                                pt[:], wt[:, ks, hb3 * P:(hb3 + 1) * P], hT[:, ks],
                                start=(ks == 0), stop=(ks == KSP - 1))
                        nc.any.tensor_copy(qT[:, hb], pt[:])
                # ---------- K^T -> dram ----------
                for half in range(2):
                    wt = load_w("wk", l, slice(half * 384, (half + 1) * 384))
                    for hb3 in range(3):
                        hb = half * 3 + hb3
                        pt = psMM.tile([P, 2 * P], F32, tag="mm")
                        for ks in range(KSP):
                            nc.tensor.matmul(
                                pt[:], wt[:, ks, hb3 * P:(hb3 + 1) * P], hT[:, ks],
                                start=(ks == 0), stop=(ks == KSP - 1))
                        ktmp = etp.tile([P, 2 * P], F32, tag="ktmp")
                        nc.any.tensor_copy(ktmp[:], pt[:])
                        nc.sync.dma_start(
                            kT_loc.rearrange("(o p) m -> p o m", p=P)[:, hb], ktmp[:])
                # ---------- V (token-major) -> dram ----------
                for half in range(2):
                    wt = load_w("wv", l, slice(half * 384, (half + 1) * 384))
                    for t in range(2):
                        pt = psMM.tile([P, 384], F32, tag="mm")
                        for ks in range(KSP):
                            nc.tensor.matmul(
                                pt[:], hT[:, ks, t * P:(t + 1) * P], wt[:, ks],
                                start=(ks == 0), stop=(ks == KSP - 1))
                        vtmp = etp.tile([P, 384], F32, tag="vtmp")
                        nc.any.tensor_copy(vtmp[:], pt[:])
                        for hh in range(NH // 2):
                            head = half * (NH // 2) + hh
                            nc.sync.dma_start(
                                v_loc.rearrange("(t p) n d -> p t n d", p=P)[
                                    :, t, head, 0:HD],
                                vtmp[:, hh * HD:(hh + 1) * HD])

                # ---------- all-gather K^T, V ----------
                if KNO_GATHER:
                    continue
                nc.gpsimd.collective_compute(
                    "AllGather", ALU.bypass, replica_groups=GROUPS,
                    ins=[kT_loc.ap().opt()], outs=[kT_gat.ap().opt()])
                nc.gpsimd.collective_compute(
                    "AllGather", ALU.bypass, replica_groups=GROUPS,
                    ins=[v_loc.ap().opt()], outs=[v_gat.ap().opt()])
                kTg = kvp.tile([P, KSP, NCHUNK * P], F32, tag="kvbig")
                for blk in range(4):
                    nc.sync.dma_start(
                        kTg[:, 0:KS, blk * 2 * P:(blk + 1) * 2 * P],
                        kT_gat[blk].rearrange("(o p) m -> p o m", p=P))
                vg = kvp.tile([P, 4, 2, NH, HD + 1], F32, tag="vg")
                for blk in range(4):
                    for t in range(2):
                        nc.sync.dma_start(
                            vg[:, blk, t],
                            v_gat[blk].rearrange("(t p) n d -> p t n d", p=P)[:, t])

                # ---------- attention ----------
                if KNO_ATTN:
                    continue
                oT = oTp.tile([P, KSP, 2 * P], F32, tag="oT")
                nc.vector.memset(oT[:, KS], 0.0)
                nc.vector.memset(oT[0:1, KS], 1.0)
                for head in range(NH):
                    hb, hp = head // 2, (head % 2) * HD
                    ops = psOT.tile([HD + 1, 2 * P], F32, tag="ot")
                    for j in range(NCHUNK):
                        blk, sub = POS_OF[j]
                        spt = psSC.tile([P, 2 * P], F32, tag="sc")
                        nc.tensor.matmul(
                            spt[:],
                            kTg[hp:hp + HD, hb, (2 * blk + sub) * P:(2 * blk + sub + 1) * P],
                            qT[hp:hp + HD, hb], start=True, stop=True)
                        et = etp.tile([P, 2 * P], F32, tag="et")
                        for t in range(2):
                            nc.vector.scalar_tensor_tensor(
                                out=spt[:, t * P:(t + 1) * P],
                                in0=causal[:], scalar=dflag[:, j, t:t + 1],
                                in1=spt[:, t * P:(t + 1) * P],
                                op0=ALU.mult, op1=ALU.add)
                            nc.scalar.activation(
                                et[:, t * P:(t + 1) * P], spt[:, t * P:(t + 1) * P],
                                AFT.Exp, bias=ambvis[:, j, t:t + 1], scale=0.125)
                        nc.tensor.matmul(
                            ops[:], vg[:, blk, sub, head], et[:],
                            start=(j == 0), stop=(j == NCHUNK - 1))
                    dn = smallp.tile([1, 2 * P], F32, tag="dn")
                    nc.vector.reciprocal(dn[:], ops[HD:HD + 1])
                    dnb = etp.tile([HD, 2 * P], F32, tag="dnb")
                    nc.gpsimd.partition_broadcast(dnb[:], dn[:])
                    nc.vector.tensor_mul(oT[hp:hp + HD, hb], ops[0:HD], dnb[:])

                # ---------- Wo + residual ----------
                for half in range(2):
                    wt = load_w("wo", l, slice(half * 384, (half + 1) * 384))
                    for t in range(2):
                        pt = psMM.tile([P, 384], F32, tag="mm")
                        for ks in range(KSP):
                            nc.tensor.matmul(
                                pt[:], oT[:, ks, t * P:(t + 1) * P], wt[:, ks],
                                start=(ks == 0), stop=(ks == KSP - 1))
                        nc.vector.tensor_add(
                            x_sb[:, t, half * 384:(half + 1) * 384],
                            x_sb[:, t, half * 384:(half + 1) * 384], pt[:])

                # ---------- LN2 + transpose ----------
                if KNO_MLP:
                    continue
                h2T = ln_transpose()

                # ---------- Wr -> rr ----------
                rr = rrp.tile([P, 2, FF], F32, tag="rr")
                for ffc in range(FF // 384):
                    wt = load_w("wr", l, slice(ffc * 384, (ffc + 1) * 384))
                    for t in range(2):
                        pt = psMM.tile([P, 384], F32, tag="mm")
                        for ks in range(KSP):
                            nc.tensor.matmul(
                                pt[:], h2T[:, ks, t * P:(t + 1) * P], wt[:, ks],
                                start=(ks == 0), stop=(ks == KSP - 1))
                        nc.any.tensor_copy(rr[:, t, ffc * 384:(ffc + 1) * 384], pt[:])

                # ---------- top-k threshold per chunk ----------
                if KNO_TOPK:
                    continue
                los = []
                for t in range(2):
                    lo = lop.tile([P, 1], F32, tag=f"lo{t}")
                    hi = smallp.tile([P, 1], F32, tag="tk_hi")
                    mid = smallp.tile([P, 1], F32, tag="tk_mid")
                    cnt = smallp.tile([P, 1], F32, tag="tk_cnt")
                    ge = smallp.tile([P, 1], mybir.dt.uint8, tag="tk_ge")
                    le = smallp.tile([P, 1], mybir.dt.uint8, tag="tk_le")
                    scr = scrp.tile([P, FF], BF16, tag="scr")
                    nc.vector.memset(lo[:], BIS_LO)
                    nc.vector.memset(hi[:], BIS_HI)
                    for it in range(N_BISECT):
                        nc.vector.tensor_add(mid[:], lo[:], hi[:])
                        nc.vector.tensor_scalar_mul(mid[:], mid[:], 0.5)
                        nc.vector.tensor_scalar(
                            scr[:], rr[:, t], mid[:], 0.0, ALU.is_gt,
                            ALU.add, accum_out=cnt[:])
                        nc.vector.tensor_scalar(ge[:], cnt[:], float(TOPK) - 0.5,
                                                None, ALU.is_ge)
                        nc.vector.tensor_scalar(le[:], cnt[:], float(TOPK) - 0.5,
                                                None, ALU.is_le)
                        nc.vector.copy_predicated(lo[:], ge[:], mid[:])
                        nc.vector.copy_predicated(hi[:], le[:], mid[:])
                    los.append(lo)

                # ---------- W1 + gelu + mask + W2 (streamed over 4 ff blocks) ----------
                if KNO_W12:
                    continue
                w2dt = F32R if tail else F32
                ones_blk = ones_blkr if tail else ones_blk32
                w2sb = w2sbp.tile([P, 2, H], F32, tag="w2sb")
                for ffb in range(NFFB):
                    cols = slice(ffb * 768, (ffb + 1) * 768)
                    hidb = hidp.tile([P, 2, 768], F32, tag="hidb")
                    for half in range(2):
                        wt = load_w("w1", l, slice(ffb * 768 + half * 384,
                                                   ffb * 768 + (half + 1) * 384))
                        for t in range(2):
                            pt = psMM.tile([P, 384], F32, tag="mm")
                            for ks in range(KSP):
                                nc.tensor.matmul(
                                    pt[:], h2T[:, ks, t * P:(t + 1) * P], wt[:, ks],
                                    start=(ks == 0), stop=(ks == KSP - 1))
                            nc.scalar.activation(
                                hidb[:, t, half * 384:(half + 1) * 384], pt[:], AFT.Gelu)
                    for t in range(2):
                        nc.vector.scalar_tensor_tensor(
                            out=hidb[:, t], in0=rr[:, t, cols], scalar=los[t][:],
                            in1=hidb[:, t], op0=ALU.is_gt, op1=ALU.mult)
                    hidT = hidTp.tile([P, KS, 2 * P], w2dt, tag="hidT")
                    for t in range(2):
                        for ks6 in range(KS):
                            pt = psTR.tile([P, P], F32, tag="tr")
                            nc.tensor.transpose(
                                pt[:], hidb[:, t, ks6 * P:(ks6 + 1) * P], ident)
                            nc.any.tensor_copy(hidT[:, ks6, t * P:(t + 1) * P], pt[:])
                    wt2 = wst.tile([P, KS, 768], w2dt, tag="wst")
                    nc.sync.dma_start(
                        wt2[:], w_in[("w2", l)].rearrange("(o p) m -> p o m", p=P)[
                            :, ffb * KS:(ffb + 1) * KS, :])
                    for t in range(2):
                        for nh2 in range(2):
                            pt = psMM.tile([P, 384], F32, tag="mm")
                            for ks6 in range(KS):
                                nc.tensor.matmul(
                                    pt[:], hidT[:, ks6, t * P:(t + 1) * P],
                                    wt2[:, ks6, nh2 * 384:(nh2 + 1) * 384],
                                    start=(ks6 == 0), stop=(ks6 == KS - 1))
                            dst = w2sb[:, t, nh2 * 384:(nh2 + 1) * 384]
                            if ffb == 0:
                                nc.vector.tensor_copy(dst, pt[:])
                            else:
                                nc.vector.tensor_add(dst, dst, pt[:])
                # bias row of W2 (row 3072) via the ones block
                wt2b = wst.tile([P, 1, 768], w2dt, tag="wst")
                nc.sync.dma_start(
                    wt2b[:], w_in[("w2", l)].rearrange("(o p) m -> p o m", p=P)[
                        :, FFKS:FFKS + 1, :])
                for t in range(2):
                    for nh2 in range(2):
                        pt = psMM.tile([P, 384], F32, tag="mm")
                        nc.tensor.matmul(
                            pt[:], ones_blk[:, 0, t * P:(t + 1) * P],
                            wt2b[:, 0, nh2 * 384:(nh2 + 1) * 384],
                            start=True, stop=True)
                        dst = w2sb[:, t, nh2 * 384:(nh2 + 1) * 384]
                        nc.vector.tensor_add(dst, dst, pt[:])
                        nc.vector.tensor_add(
                            x_sb[:, t, nh2 * 384:(nh2 + 1) * 384],
                            x_sb[:, t, nh2 * 384:(nh2 + 1) * 384], dst)

            # ---------- final LN + gather + LM head ----------
            xfT = ln_transpose(tag_dt=F32R)
            nc.sync.dma_start(
                xfT_loc.rearrange("(o p) m -> p o m", p=P),
                xfT[:, 0:KS, :])
            nc.gpsimd.collective_compute(
                "AllGather", ALU.bypass, replica_groups=GROUPS,
                ins=[xfT_loc.ap().opt()], outs=[xfT_gat.ap().opt()])
            XT = kvp.tile([P, KSP, NCHUNK * P], F32R, tag="kvbig")
            for blk in range(4):
                nc.vector.tensor_copy(XT[:, KS, blk * 2 * P:(blk + 1) * 2 * P],
                                      ones_blk32[:, 0])
            for blk in range(4):
                nc.sync.dma_start(
                    XT[:, 0:KS, blk * 2 * P:(blk + 1) * 2 * P],
                    xfT_gat[blk].rearrange("(o p) m -> p o m", p=P))
            NC_LM = 16
            LMW = VS // NC_LM  # 500
            for nchunk in range(NC_LM):
                wt = wst.tile([P, KSP, LMW], F32R, tag="wst")
                nc.sync.dma_start(
                    wt[:], wlm_d.rearrange("(o p) m -> p o m", p=P)[
                        :, :, nchunk * LMW:(nchunk + 1) * LMW])
                for tt in range(NCHUNK):
                    pt = psMM.tile([P, LMW], F32, tag="mm")
                    for ks in range(KSP):
                        nc.tensor.matmul(
                            pt[:], XT[:, ks, tt * P:(tt + 1) * P], wt[:, ks],
                            start=(ks == 0), stop=(ks == KSP - 1))
                    ot = etp.tile([P, LMW], F32, tag="lmout")
                    nc.any.tensor_copy(ot[:], pt[:])
                    nc.sync.dma_start(
                        out_d[tt * P:(tt + 1) * P, nchunk * LMW:(nchunk + 1) * LMW],
                        ot[:])

    nc.compile()
    return nc


_CACHE = {}


def _get_nc():
    if "nc" not in _CACHE:
        _CACHE["nc"] = _build_nc()
    return _CACHE["nc"]


def _pad_w(w, bias, gain=None, ln_b=None):
    """Fold LN gain/bias + linear bias into a padded weight [K+128, N]."""
    K, N = w.shape
    out = np.zeros((K + P, N), np.float32)
    wg = w * gain[:, None] if gain is not None else w
    out[:K] = wg
    brow = bias.astype(np.float64).copy()
    if ln_b is not None:
        brow = brow + ln_b.astype(np.float64) @ wg.astype(np.float64)
    out[K] = brow.astype(np.float32)
    return out


def kernel(**inputs):
    ids = np.asarray(inputs["input_ids"])
    amask = np.asarray(inputs["attention_mask"]).astype(np.float32)
    tok = np.asarray(inputs["tok_emb"], np.float32)
    pos = np.asarray(inputs["pos_emb"], np.float32)
    lp = {k: np.asarray(v, np.float32) for k, v in inputs["layer_params"].items()}
    lnf_g = np.asarray(inputs["lnf_g"], np.float32)
    lnf_b = np.asarray(inputs["lnf_b"], np.float32)
    Wlm = np.asarray(inputs["Wlm"], np.float32)
    blm = np.asarray(inputs["blm"], np.float32)

    nc = _get_nc()

    x0 = tok[ids] + pos[None, :S]
    amb = (amask - 1.0) * 1e4          # [2, 1024]: 0 attend / -1e4 pad

    wmaps = {}
    for l in range(L):
        g1, b1 = lp["ln1_g"][l], lp["ln1_b"][l]
        g2, b2 = lp["ln2_g"][l], lp["ln2_b"][l]
        wmaps[f"wq{l}"] = _pad_w(lp["Wq"][l], lp["bq"][l], g1, b1)
        wmaps[f"wk{l}"] = _pad_w(lp["Wk"][l], lp["bk"][l], g1, b1)
        wmaps[f"wv{l}"] = _pad_w(lp["Wv"][l], lp["bv"][l], g1, b1)
        wmaps[f"wo{l}"] = _pad_w(lp["Wo"][l], lp["bo"][l])
        wmaps[f"w1{l}"] = _pad_w(lp["W1"][l], lp["b1"][l], g2, b2)
        wmaps[f"wr{l}"] = _pad_w(lp["Wr"][l], lp["br"][l], g2, b2)
        wmaps[f"w2{l}"] = _pad_w(lp["W2"][l], lp["b2"][l])

    in_maps = []
    for core in range(NCORE):
        g, c = core // 4, core % 4
        own = CHUNK_OF[c]
        x0c = np.stack([x0[g, own[0] * P:(own[0] + 1) * P],
                        x0[g, own[1] * P:(own[1] + 1) * P]])
        ambvis = np.zeros((P, NCHUNK, 2), np.float32)
        dflag = np.zeros((P, NCHUNK, 2), np.float32)
        for t in range(2):
            for j in range(NCHUNK):
                ambvis[:, j, t] = amb[g, j * P:(j + 1) * P]
                if j > own[t]:
                    ambvis[:, j, t] += -1e4
                elif j == own[t]:
                    dflag[:, j, t] = 1.0
        m = dict(wmaps)
        m["x0"] = np.ascontiguousarray(x0c)
        m["ambvis"] = ambvis
        m["dflag"] = dflag
        m["wlm"] = _pad_w(Wlm[:, c * VS:(c + 1) * VS], blm[c * VS:(c + 1) * VS],
                          lnf_g, lnf_b)
        in_maps.append(m)

    res = run_bass_kernel_spmd(nc, in_maps, core_ids=list(range(NCORE)))
    kernel.last_in_maps = in_maps

    out = np.zeros((B, S, V), np.float32)
    for core in range(NCORE):
        g, c = core // 4, core % 4
        lg = res.results[core]["logits"]
        for blk in range(4):
            ca, cb = CHUNK_OF[blk]
            out[g, ca * P:(ca + 1) * P, c * VS:(c + 1) * VS] = \
                lg[blk * 2 * P: blk * 2 * P + P]
            out[g, cb * P:(cb + 1) * P, c * VS:(c + 1) * VS] = \
                lg[blk * 2 * P + P: (blk + 1) * 2 * P]
    return out


def bench_exec_ns(n_iters=3):
    """Time pure device execution (inputs pre-transferred) of the last run."""
    import time
    import jax
    from jax.sharding import Mesh, PartitionSpec, NamedSharding
    from jax.experimental.shard_map import shard_map
    from concourse import bass2jax
    from concourse.bass2jax import _bass_exec_p, partition_id_tensor
    import concourse.mybir as mb

    nc = _get_nc()
    in_maps = kernel.last_in_maps
    n_cores = NCORE
    bass2jax.install_neuronx_cc_hook()

    in_names, out_names, out_avals, zero_outs = [], [], [], []
    partition_name = nc.partition_id_tensor.name if nc.partition_id_tensor else None
    for alloc in nc.m.functions[0].allocations:
        if not isinstance(alloc, mb.MemoryLocationSet):
            continue
        name = alloc.memorylocations[0].name
        if alloc.kind == "ExternalInput":
            if name != partition_name:
                in_names.append(name)
        elif alloc.kind == "ExternalOutput":
            shape = tuple(alloc.tensor_shape)
            dt = mb.dt.np(alloc.dtype)
            out_avals.append(jax.core.ShapedArray(shape, dt))
            out_names.append(name)
            zero_outs.append(np.zeros(shape, dt))
    n_params = len(in_names)
    n_outs = len(out_names)
    all_in_names = list(in_names) + list(out_names)
    if partition_name is not None:
        all_in_names.append(partition_name)
    donate = tuple(range(n_params, n_params + n_outs))

    def _body(*args):
        operands = list(args)
        if partition_name is not None:
            operands.append(partition_id_tensor())
        outs = _bass_exec_p.bind(
            *operands, out_avals=tuple(out_avals), in_names=tuple(all_in_names),
            out_names=tuple(out_names), lowering_input_output_aliases=(),
            sim_require_finite=True, sim_require_nnan=True, nc=nc)
        return tuple(outs)

    devices = jax.devices()[:n_cores]
    mesh = Mesh(np.asarray(devices), ("core",))
    spec = NamedSharding(mesh, PartitionSpec("core"))
    sharded = jax.jit(
        shard_map(_body, mesh=mesh, in_specs=(PartitionSpec("core"),) * (n_params + n_outs),
                  out_specs=(PartitionSpec("core"),) * n_outs, check_rep=False),
        donate_argnums=donate, keep_unused=True)
    concat_in = [np.concatenate([np.asarray(m[name]) for m in in_maps], axis=0)
                 for name in in_names]
    dev_in = [jax.device_put(a, spec) for a in concat_in]
    for a in dev_in:
        a.block_until_ready()
    times = []
    for _ in range(n_iters):
        dev_zeros = [jax.device_put(
            np.zeros((n_cores * z.shape[0], *z.shape[1:]), z.dtype), spec)
            for z in zero_outs]
        for a in dev_zeros:
            a.block_until_ready()
        t0 = time.perf_counter()
        outs = sharded(*dev_in, *dev_zeros)
        for o in outs:
            o.block_until_ready()
        t1 = time.perf_counter()
        times.append(t1 - t0)
        del outs
    return int(min(times) * 1e9)
